# Initial kernel scaffold
#
"""Trainium2 Bass kernel for nn_Interaction (sparse_attention).

Computation (B=16, L=1024, E=256, NH=2, H=128):
  d = SelfAttn(x_d; wkx_d, wqx_d, Wp_d, bp_d)   # sigmoid-feature attention
  p = SelfAttn(x_p; ...)
  out = MAttn(k=d, q=p; wkx_m, wqx_m, Wm, bm)   # [B, 1, 256]

Sharding: data-parallel over batch across 8 cores (2 batches/core), weights
replicated. Each core computes its 2 batches fully; host concatenates.

Layout strategy (per core, per batch, per block):
  - load x [1024,256] natural, PE-transpose to xT [256p, 1024]
  - kxT/qxT = sigmoid(w.T @ xT) in [H=128p, L] layout (PE + ACT)
  - per head: S^T chunks [k=128p, q=1024] = kxT_chunk.T @ qxT (PE)
    expS = exp(S^T * 1/sqrt(H)) (ACT, PSUM->SBUF)
    sums[q] via ones[128,128] lhsT matmul -> [128p, q] (replicated rows)
    U^T [f=128p, q] += kx_chunk.T-free... lhsT=kx (PE, accum over k-chunks)
  - normalize U^T by 1/sums (DVE reciprocal + mul), proj to final^T [o, l]
    = d^T directly feeds m-attention (no extra transpose)
  - m-attn: kxmT/qxmT [f,l] via PE, memory^T (DVE), s-row [1,L] via Wm-lhsT
    matmul, sigmoid+exp (ACT accum_out gives total sum), broadcast via DRAM,
    final reduce via DVE tensor_tensor_reduce.
All matmuls run in float32r (full-rate fp32 PE mode; bits are fp32).
"""

import math
import os
import sys
from contextlib import ExitStack

import numpy as np

if "/opt/trn_rl_repo" not in sys.path:
    sys.path.insert(0, "/opt/trn_rl_repo")

import concourse.bass as bass
import concourse.tile as tile
from concourse import bacc
from concourse import mybir
from concourse.bass_utils import run_bass_kernel_spmd
from concourse.masks import make_identity

F32 = mybir.dt.float32
MM_DT = mybir.dt.float32r  # flip to mybir.dt.float32 if numerics demand
AF = mybir.ActivationFunctionType
OP = mybir.AluOpType

B, L, E, NH, H, OUT, HM = 16, 1024, 256, 2, 128, 256, 256
CORES, BPC = 8, 2
SCALE = 1.0 / math.sqrt(H)
MSCALE = 1.0 / math.sqrt(HM)

WNAMES = [
    "wkx_d", "wqx_d", "Wp_d", "bp_d",
    "wkx_p", "wqx_p", "Wp_p", "bp_p",
    "wkx_m", "wqx_m", "Wm", "bm",
]


def _f32(ap):
    """View a float32r AP as plain fp32 (bits identical after rounding)."""
    if ap.dtype is F32:
        return ap
    return ap.bitcast(F32)


def _bcast_dram(ap, parts):
    """Partition-broadcast AP over a DRAM region (step-0 partition dim)."""
    return bass.AP(tensor=ap.tensor, offset=ap.offset, ap=[[0, parts]] + list(ap.ap[1:]))


def build():
    nc = bacc.Bacc()

    x_in = {
        "d": nc.dram_tensor("x_d", [BPC, L, E], F32, kind="ExternalInput"),
        "p": nc.dram_tensor("x_p", [BPC, L, E], F32, kind="ExternalInput"),
    }
    wkx = {
        "d": nc.dram_tensor("wkx_d", [NH, E, H], F32, kind="ExternalInput"),
        "p": nc.dram_tensor("wkx_p", [NH, E, H], F32, kind="ExternalInput"),
    }
    wqx = {
        "d": nc.dram_tensor("wqx_d", [NH, E, H], F32, kind="ExternalInput"),
        "p": nc.dram_tensor("wqx_p", [NH, E, H], F32, kind="ExternalInput"),
    }
    Wp = {
        "d": nc.dram_tensor("Wp_d", [NH * H, OUT], F32, kind="ExternalInput"),
        "p": nc.dram_tensor("Wp_p", [NH * H, OUT], F32, kind="ExternalInput"),
    }
    bp = {
        "d": nc.dram_tensor("bp_d", [OUT], F32, kind="ExternalInput"),
        "p": nc.dram_tensor("bp_p", [OUT], F32, kind="ExternalInput"),
    }
    wkx_m = nc.dram_tensor("wkx_m", [E, HM], F32, kind="ExternalInput")
    wqx_m = nc.dram_tensor("wqx_m", [E, HM], F32, kind="ExternalInput")
    Wm = nc.dram_tensor("Wm", [HM, 1], F32, kind="ExternalInput")
    bm = nc.dram_tensor("bm", [1], F32, kind="ExternalInput")
    out_t = nc.dram_tensor("out", [BPC, 1, OUT], F32, kind="ExternalOutput")

    with tile.TileContext(nc) as tc, ExitStack() as ctx:
        consts = ctx.enter_context(tc.tile_pool(name="consts", bufs=1))
        xload = ctx.enter_context(tc.tile_pool(name="xload", bufs=2))
        xTp = ctx.enter_context(tc.tile_pool(name="xTp", bufs=2))
        kqp = ctx.enter_context(tc.tile_pool(name="kqp", bufs=5))
        kxp = ctx.enter_context(tc.tile_pool(name="kxp", bufs=2))
        expp = ctx.enter_context(tc.tile_pool(name="expp", bufs=3))
        nrmp = ctx.enter_context(tc.tile_pool(name="nrmp", bufs=2))
        selfout = ctx.enter_context(tc.tile_pool(name="selfout", bufs=8))
        matp = ctx.enter_context(tc.tile_pool(name="matp", bufs=5))
        smallp = ctx.enter_context(tc.tile_pool(name="smallp", bufs=2))
        # PSUM: 8 banks total. psS 2x[128,1024]=4, psU 1x=2, psV 1x=2.
        psS = ctx.enter_context(tc.tile_pool(name="psS", bufs=2, space="PSUM"))
        psU = ctx.enter_context(tc.tile_pool(name="psU", bufs=1, space="PSUM"))
        psV = ctx.enter_context(tc.tile_pool(name="psV", bufs=1, space="PSUM"))

        ident = consts.tile([128, 128], F32)
        make_identity(nc, ident[:])
        ones_raw = consts.tile([128, 128], F32, tag="ones_raw")
        nc.vector.memset(ones_raw[:], 1.0)
        ones128 = consts.tile([128, 128], MM_DT)
        nc.vector.tensor_copy(ones128[:], ones_raw[:])

        def round_w(tag, shape, src_ap):
            raw = consts.tile(shape, F32, tag=tag + "_raw")
            nc.gpsimd.dma_start(raw[:], src_ap)
            wr = consts.tile(shape, MM_DT, tag=tag)
            nc.vector.tensor_copy(wr[:], raw[:])
            return wr

        # ---- weights to SBUF ----
        w_sb = {}  # (blk, n, kq, ec) -> [128, H] (lhsT for prep)
        for blk in ("d", "p"):
            for n in range(NH):
                for kq, t in (("k", wkx[blk]), ("q", wqx[blk])):
                    for ec in range(2):
                        w_sb[blk, n, kq, ec] = round_w(
                            f"w{blk}{n}{kq}{ec}", [128, H],
                            t[n, ec * 128:(ec + 1) * 128, :])
        Wp_sb = {}  # (blk, fc) -> [128, OUT]
        bp_sb = {}  # (blk, oc) -> [128, 1]
        for blk in ("d", "p"):
            for fc in range(2):
                Wp_sb[blk, fc] = round_w(
                    f"Wp{blk}{fc}", [128, OUT], Wp[blk][fc * 128:(fc + 1) * 128, :])
            for oc in range(2):
                bt = consts.tile([128, 1], F32, tag=f"bp{blk}{oc}")
                nc.gpsimd.dma_start(
                    bt[:],
                    bp[blk][oc * 128:(oc + 1) * 128].rearrange("(p o) -> p o", o=1),
                )
                bp_sb[blk, oc] = bt
        wkm_sb, wqm_sb = {}, {}
        for ec in range(2):
            wkm_sb[ec] = round_w(f"wkm{ec}", [128, HM], wkx_m[ec * 128:(ec + 1) * 128, :])
            wqm_sb[ec] = round_w(f"wqm{ec}", [128, HM], wqx_m[ec * 128:(ec + 1) * 128, :])
        Wm_col = {}
        for fc in range(2):
            Wm_col[fc] = round_w(f"Wm{fc}", [128, 1], Wm[fc * 128:(fc + 1) * 128, :])
        bm_sb = consts.tile([1, 1], F32, tag="bm")
        nc.gpsimd.dma_start(bm_sb[:], bm.rearrange("(p o) -> p o", o=1))

        so_tiles = {}  # (blk, b, oc) -> final^T [128, L]

        def self_attn(blk, b):
            # load x natural [p, lc, e]
            xt = xload.tile([128, 8, E], F32, tag="x")
            nc.gpsimd.dma_start(
                xt[:], x_in[blk][b].rearrange("(lc p) e -> p lc e", p=128)
            )
            # transpose to xT [e-part, ec, l]
            xT = xTp.tile([128, 2, L], MM_DT, tag="xT")
            for lc in range(8):
                for ec in range(2):
                    pt = psS.tile([128, 128], F32, tag="S")
                    nc.tensor.transpose(
                        pt[:], xt[:, lc, ec * 128:(ec + 1) * 128], ident[:]
                    )
                    nc.vector.tensor_copy(xT[:, ec, lc * 128:(lc + 1) * 128], pt[:])
            # prep: kxT/qxT = sigmoid(w.T @ xT) per head
            kqT = {}
            for n in range(NH):
                for kq in ("k", "q"):
                    ps = psS.tile([128, L], F32, tag="S")
                    for qc in range(2):
                        for ec in range(2):
                            nc.tensor.matmul(
                                ps[:, qc * 512:(qc + 1) * 512],
                                lhsT=(w_sb[blk, n, kq, ec][:]),
                                rhs=(xT[:, ec, qc * 512:(qc + 1) * 512]),
                                start=(ec == 0),
                                stop=(ec == 1),
                            )
                    t = kqp.tile([128, L], MM_DT, tag="kq")
                    nc.scalar.activation(t[:], ps[:], AF.Sigmoid)
                    kqT[n, kq] = t
            # attention per head
            normU = {}
            for n in range(NH):
                kx_n = kxp.tile([128, 8, H], MM_DT, tag="kx")
                for kc in range(8):
                    pt = psS.tile([128, 128], F32, tag="S")
                    nc.tensor.transpose(
                        pt[:], _f32(kqT[n, "k"][:, kc * 128:(kc + 1) * 128]), ident[:]
                    )
                    nc.vector.tensor_copy(kx_n[:, kc, :], pt[:])
                pU = psU.tile([128, L], F32, tag="U")
                pSum = psV.tile([128, L], F32, tag="V")
                for kc in range(8):
                    pS = psS.tile([128, L], F32, tag="S")
                    for qc in range(2):
                        nc.tensor.matmul(
                            pS[:, qc * 512:(qc + 1) * 512],
                            lhsT=(kqT[n, "k"][:, kc * 128:(kc + 1) * 128]),
                            rhs=(kqT[n, "q"][:, qc * 512:(qc + 1) * 512]),
                            start=True,
                            stop=True,
                        )
                    ex = expp.tile([128, L], MM_DT, tag="exp")
                    nc.scalar.activation(ex[:], pS[:], AF.Exp, scale=SCALE)
                    for qc in range(2):
                        sl = slice(qc * 512, (qc + 1) * 512)
                        nc.tensor.matmul(
                            pSum[:, sl], lhsT=(ones128[:]), rhs=(ex[:, sl]),
                            start=(kc == 0), stop=(kc == 7), skip_group_check=True,
                        )
                        nc.tensor.matmul(
                            pU[:, sl], lhsT=(kx_n[:, kc, :]), rhs=(ex[:, sl]),
                            start=(kc == 0), stop=(kc == 7), skip_group_check=True,
                        )
                rec = nrmp.tile([128, L], F32, tag="rec")
                nc.vector.reciprocal(rec[:], pSum[:])
                nU = nrmp.tile([128, L], MM_DT, tag="nU")
                nc.vector.tensor_mul(nU[:], pU[:], rec[:])
                normU[n] = nU
            # proj: final^T [o-part, l]
            for oc in range(2):
                pF = psU.tile([128, L], F32, tag="U")
                for qc in range(2):
                    for fc in range(2):
                        nc.tensor.matmul(
                            pF[:, qc * 512:(qc + 1) * 512],
                            lhsT=(Wp_sb[blk, fc][:, oc * 128:(oc + 1) * 128]),
                            rhs=(normU[fc][:, qc * 512:(qc + 1) * 512]),
                            start=(fc == 0),
                            stop=(fc == 1),
                        )
                so = selfout.tile([128, L], MM_DT, tag="so")
                nc.vector.tensor_scalar(
                    out=so[:], in0=pF[:], scalar1=bp_sb[blk, oc][:],
                    scalar2=None, op0=OP.add,
                )
                so_tiles[blk, b, oc] = so

        def m_attn(b):
            d_t = (so_tiles["d", b, 0], so_tiles["d", b, 1])
            p_t = (so_tiles["p", b, 0], so_tiles["p", b, 1])
            kmT = {}
            for tname, srcs, wsb in (("k", d_t, wkm_sb), ("q", p_t, wqm_sb)):
                for fc in range(2):
                    ps = psS.tile([128, L], F32, tag="S")
                    for lc2 in range(2):
                        for ec in range(2):
                            nc.tensor.matmul(
                                ps[:, lc2 * 512:(lc2 + 1) * 512],
                                lhsT=(wsb[ec][:, fc * 128:(fc + 1) * 128]),
                                rhs=(srcs[ec][:, lc2 * 512:(lc2 + 1) * 512]),
                                start=(ec == 0),
                                stop=(ec == 1),
                            )
                    km = matp.tile([128, L], MM_DT, tag="mat")
                    nc.scalar.activation(km[:], ps[:], AF.Sigmoid)
                    kmT[tname, fc] = km
            # memory^T = kxT * qxT; s-row = Wm.T @ memory^T
            pRow = psV.tile([128, L], F32, tag="V")
            for fc in range(2):
                mm = kmT["k", fc]
                nc.vector.tensor_mul(mm[:], _f32(kmT["k", fc][:]), _f32(kmT["q", fc][:]))
                for lc2 in range(2):
                    nc.tensor.matmul(
                        pRow[0:1, lc2 * 512:(lc2 + 1) * 512],
                        lhsT=(Wm_col[fc][:]),
                        rhs=(mm[:, lc2 * 512:(lc2 + 1) * 512]),
                        start=(fc == 0),
                        stop=(fc == 1),
                        skip_group_check=True,
                    )
            sig_row = smallp.tile([1, L], F32, tag="sig")
            nc.scalar.activation(sig_row[:], pRow[0:1, :], AF.Sigmoid, bias=bm_sb[:])
            exp_row = smallp.tile([1, L], F32, tag="exp")
            nc.scalar.activation(exp_row[:], sig_row[:], AF.Exp, scale=MSCALE)
            T11 = smallp.tile([1, 1], F32, tag="t11")
            nc.vector.reduce_sum(T11[:], exp_row[:], axis=mybir.AxisListType.X)
            # K=1 ones-matmuls broadcast the row / total across 128 partitions
            pB = psS.tile([128, L], F32, tag="S")
            for lc2 in range(2):
                nc.tensor.matmul(
                    pB[:, lc2 * 512:(lc2 + 1) * 512], lhsT=ones_raw[0:1, :],
                    rhs=exp_row[0:1, lc2 * 512:(lc2 + 1) * 512],  # fp32 K=1
                    start=True, stop=True,
                )
            pT = psV.tile([128, L], F32, tag="V")
            nc.tensor.matmul(
                pT[:, 0:1], lhsT=ones_raw[0:1, :], rhs=T11[0:1, 0:1],  # fp32 K=1
                start=True, stop=True,
            )
            rT = smallp.tile([128, 1], F32, tag="rt")
            nc.vector.reciprocal(rT[:], pT[:, 0:1])
            for oc in range(2):
                junk = nrmp.tile([128, L], F32, tag="rec")
                colsum = smallp.tile([128, 1], F32, tag="cs")
                nc.vector.tensor_mul(junk[:], _f32(d_t[oc][:]), pB[:])
                nc.vector.reduce_sum(colsum[:], junk[:], axis=mybir.AxisListType.X)
                cn = smallp.tile([128, 1], F32, tag="cn")
                nc.vector.tensor_scalar(
                    out=cn[:], in0=colsum[:], scalar1=rT[:], scalar2=None,
                    op0=OP.mult,
                )
                nc.gpsimd.dma_start(out_t[b, 0:1, oc * 128:(oc + 1) * 128], cn[:])

        for b in range(BPC):
            for blk in ("d", "p"):
                self_attn(blk, b)
            m_attn(b)

    nc.compile()
    return nc


_CACHE = {}


def _get_nc():
    if "nc" not in _CACHE:
        _CACHE["nc"] = build()
    return _CACHE["nc"]


def _make_in_maps(inputs):
    in_maps = []
    for c in range(CORES):
        m = {}
        sl = slice(c * BPC, (c + 1) * BPC)
        m["x_d"] = np.ascontiguousarray(np.asarray(inputs["x_d"], np.float32)[sl])
        m["x_p"] = np.ascontiguousarray(np.asarray(inputs["x_p"], np.float32)[sl])
        for name in WNAMES:
            m[name] = np.ascontiguousarray(np.asarray(inputs[name], np.float32))
        in_maps.append(m)
    return in_maps


def run_spmd(inputs, **kw):
    nc = _get_nc()
    res = run_bass_kernel_spmd(nc, _make_in_maps(inputs), core_ids=list(range(CORES)), **kw)
    out = np.concatenate([r["out"] for r in res.results], axis=0)
    return out, res


def kernel(**inputs):
    out, _ = run_spmd(inputs)
    return out



# revision 15
# speedup vs baseline: 1.8787x; 1.8787x over previous
"""Trainium2 Bass kernel for nn_Interaction (sparse_attention).

Computation (B=16, L=1024, E=256, NH=2, H=128):
  d = SelfAttn(x_d; wkx_d, wqx_d, Wp_d, bp_d)   # sigmoid-feature attention
  p = SelfAttn(x_p; ...)
  out = MAttn(k=d, q=p; wkx_m, wqx_m, Wm, bm)   # [B, 1, 256]

Sharding: data-parallel over batch across 8 cores (2 batches/core), weights
replicated. Each core computes its 2 batches fully; host concatenates.

v2 performance notes (vs fp32r baseline, 378us):
  - all matmul operands bf16: enables FWL (hidden LDWEIGHTS; disabled for
    fp32 "HIGH" mode), 1024-wide moving operand (fp32 caps at 512), and
    1-pass transposes.
  - softmax denominators: exp chunks accumulated on DVE (bf16 adds), one
    ones-matmul per head for the partition reduction, then
    reciprocal_approx_fast (the baseline's InstReciprocal on [128,1024]
    cost 5.3us each and stalled the PE 8x6.3us).
  - x DMAs issued before weight DMAs (baseline idled PE 27us at start),
    batch-1 front of pipeline emitted before batch-0 m-attn tail so the
    PE queue never parks behind the serial m-attn chain (also keeps HAM
    at 8/8).
  - ACT functions grouped (sigmoid phase / exp phase) to cut table loads.
"""

import math
import sys
from contextlib import ExitStack

import numpy as np

if "/opt/trn_rl_repo" not in sys.path:
    sys.path.insert(0, "/opt/trn_rl_repo")

import concourse.bass as bass
import concourse.tile as tile
from concourse import bacc
from concourse import mybir
from concourse.bass_utils import run_bass_kernel_spmd
from concourse.masks import make_identity

F32 = mybir.dt.float32
BF = mybir.dt.bfloat16
AF = mybir.ActivationFunctionType
OP = mybir.AluOpType

B, L, E, NH, H, OUT, HM = 16, 1024, 256, 2, 128, 256, 256
CORES, BPC = 8, 2
SCALE = 1.0 / math.sqrt(H)
MSCALE = 1.0 / math.sqrt(HM)

# HW-construct toggles (bisect aids; all True = fastest)
USE_FAST_RECIP = True
USE_TTR = False
BF16_TRANSPOSE = True

WNAMES = [
    "wkx_d", "wqx_d", "Wp_d", "bp_d",
    "wkx_p", "wqx_p", "Wp_p", "bp_p",
    "wkx_m", "wqx_m", "Wm", "bm",
]


def build():
    nc = bacc.Bacc()

    x_in = {
        "d": nc.dram_tensor("x_d", [BPC, L, E], F32, kind="ExternalInput"),
        "p": nc.dram_tensor("x_p", [BPC, L, E], F32, kind="ExternalInput"),
    }
    wkx = {
        "d": nc.dram_tensor("wkx_d", [NH, E, H], F32, kind="ExternalInput"),
        "p": nc.dram_tensor("wkx_p", [NH, E, H], F32, kind="ExternalInput"),
    }
    wqx = {
        "d": nc.dram_tensor("wqx_d", [NH, E, H], F32, kind="ExternalInput"),
        "p": nc.dram_tensor("wqx_p", [NH, E, H], F32, kind="ExternalInput"),
    }
    Wp = {
        "d": nc.dram_tensor("Wp_d", [NH * H, OUT], F32, kind="ExternalInput"),
        "p": nc.dram_tensor("Wp_p", [NH * H, OUT], F32, kind="ExternalInput"),
    }
    bp = {
        "d": nc.dram_tensor("bp_d", [OUT], F32, kind="ExternalInput"),
        "p": nc.dram_tensor("bp_p", [OUT], F32, kind="ExternalInput"),
    }
    wkx_m = nc.dram_tensor("wkx_m", [E, HM], F32, kind="ExternalInput")
    wqx_m = nc.dram_tensor("wqx_m", [E, HM], F32, kind="ExternalInput")
    Wm = nc.dram_tensor("Wm", [HM, 1], F32, kind="ExternalInput")
    bm = nc.dram_tensor("bm", [1], F32, kind="ExternalInput")
    out_t = nc.dram_tensor("out", [BPC, 1, OUT], F32, kind="ExternalOutput")

    with tile.TileContext(nc) as tc, ExitStack() as ctx:
        consts = ctx.enter_context(tc.tile_pool(name="consts", bufs=1))
        stage = ctx.enter_context(tc.tile_pool(name="stage", bufs=2))
        xload = ctx.enter_context(tc.tile_pool(name="xload", bufs=2))
        xbp = ctx.enter_context(tc.tile_pool(name="xbp", bufs=2))
        xTp = ctx.enter_context(tc.tile_pool(name="xTp", bufs=2))
        kqp = ctx.enter_context(tc.tile_pool(name="kqp", bufs=8))
        kxp = ctx.enter_context(tc.tile_pool(name="kxp", bufs=4))
        expp = ctx.enter_context(tc.tile_pool(name="expp", bufs=4))
        accp = ctx.enter_context(tc.tile_pool(name="accp", bufs=2))
        nrmp = ctx.enter_context(tc.tile_pool(name="nrmp", bufs=2))
        nUp = ctx.enter_context(tc.tile_pool(name="nUp", bufs=4))
        selfout = ctx.enter_context(tc.tile_pool(name="selfout", bufs=8))
        matp = ctx.enter_context(tc.tile_pool(name="matp", bufs=4))
        smallp = ctx.enter_context(tc.tile_pool(name="smallp", bufs=6))
        # PSUM: psS 2x[128,1024]f32 = 4 banks + psU 2x = 4 banks -> 8 total.
        # (softmax sums / m-attn rows ride the psS rotation)
        psS = ctx.enter_context(tc.tile_pool(name="psS", bufs=2, space="PSUM"))
        psU = ctx.enter_context(tc.tile_pool(name="psU", bufs=2, space="PSUM"))

        # ---- x DMAs for batch 0 go first: the PE's first work (transposes)
        # depends only on x, so weights can stream in behind them.
        xt_tiles = {}

        def issue_xload(blk, b, half):
            t = xload.tile([128, 4, E], F32, tag="x")
            nc.gpsimd.dma_start(
                t[:],
                x_in[blk][b, half * 512:(half + 1) * 512].rearrange(
                    "(lc p) e -> p lc e", p=128),
            )
            xt_tiles[blk, b, half] = t

        issue_xload("d", 0, 0)
        issue_xload("d", 0, 1)

        ones_f = consts.tile([128, 128], F32, tag="ones_f")
        nc.vector.memset(ones_f[:], 1.0)
        ident_f = consts.tile([128, 128], F32, tag="ident_f")
        make_identity(nc, ident_f[:])
        ident = consts.tile([128, 128], BF, tag="ident")
        nc.vector.tensor_copy(ident[:], ident_f[:])
        ones_bf = consts.tile([128, 128], BF, tag="ones_bf")
        nc.vector.tensor_copy(ones_bf[:], ones_f[:])

        xb_tiles = {}

        def cast_x(blk, b, eng=None):
            eng = eng or nc.vector
            for half in range(2):
                xb = xbp.tile([128, 4, E], BF, tag="xb")
                eng.tensor_copy(xb[:], xt_tiles[blk, b, half][:])
                xb_tiles[blk, b, half] = xb

        def load_w(tag, src_ap, shape):
            st = stage.tile(shape, F32, tag="wstage")
            nc.gpsimd.dma_start(st[:], src_ap)
            wb = consts.tile(shape, BF, tag=tag)
            nc.vector.tensor_copy(wb[:], st[:])
            return wb

        if BF16_TRANSPOSE:
            cast_x("d", 0)

        # d-block prep weights right behind x_d; x_p next; rest after.
        # One DMA per weight tensor: fewer DMA queues -> shorter final drain.
        wkq = {}   # (blk, 'k'/'q') -> [128, NH*2, H] bf16
        for kq, t in (("k", wkx["d"]), ("q", wqx["d"])):
            wkq["d", kq] = load_w(
                f"wd{kq}", t.rearrange("n (ec p) h -> p (n ec) h", p=128),
                [128, NH * 2, H])
        issue_xload("p", 0, 0)
        issue_xload("p", 0, 1)
        if BF16_TRANSPOSE:
            cast_x("p", 0)
        for kq, t in (("k", wkx["p"]), ("q", wqx["p"])):
            wkq["p", kq] = load_w(
                f"wp{kq}", t.rearrange("n (ec p) h -> p (n ec) h", p=128),
                [128, NH * 2, H])
        w_sb = {
            (blk, n, kq, ec): wkq[blk, kq][:, n * 2 + ec]
            for blk in ("d", "p") for n in range(NH)
            for kq in ("k", "q") for ec in range(2)
        }

        Wp_sb, bp_sb = {}, {}
        for blk in ("d", "p"):
            wp4 = load_w(f"Wp{blk}", Wp[blk].rearrange("(fc p) o -> p fc o", p=128),
                         [128, 2, OUT])
            for fc in range(2):
                Wp_sb[blk, fc] = wp4[:, fc]
            bt = consts.tile([128, 2], F32, tag=f"bp{blk}")
            nc.gpsimd.dma_start(
                bt[:], bp[blk].rearrange("(oc p) -> p oc", p=128))
            for oc in range(2):
                bp_sb[blk, oc] = bt[:, oc:oc + 1]
        wkm4 = load_w("wkm", wkx_m.rearrange("(ec p) f -> p ec f", p=128),
                      [128, 2, HM])
        wqm4 = load_w("wqm", wqx_m.rearrange("(ec p) f -> p ec f", p=128),
                      [128, 2, HM])
        wkm_sb = {ec: wkm4[:, ec] for ec in range(2)}
        wqm_sb = {ec: wqm4[:, ec] for ec in range(2)}
        wm2 = load_w("Wm", Wm.rearrange("(fc p) o -> p fc o", p=128), [128, 2, 1])
        Wm_col = {fc: wm2[:, fc] for fc in range(2)}
        bm_sb = consts.tile([1, 1], F32, tag="bm")
        nc.gpsimd.dma_start(bm_sb[:], bm.rearrange("(p o) -> p o", o=1))

        # ---- stages ----
        xT_tiles = {}   # (blk) -> [128, 2, L] bf16 (x^T, e on partitions)
        kqT = {}        # (blk, n, kq) -> [128, L] bf16 (sigmoid features^T)
        kx_t = {}       # (blk, n) -> [128, L] bf16 (features natural: k rows)
        normU = {}      # (blk, n) -> [128, L] bf16
        so_t = {}       # (blk, b, oc) -> [128, L] bf16 (attn output^T)
        kmT = {}

        def transpose_quad(dst, srcs, f32=False):
            pq = psS.tile([128, 512], F32 if f32 else BF, tag="S")
            idn = ident_f if f32 else ident
            for i, s in enumerate(srcs):
                nc.tensor.transpose(pq[:, i * 128:(i + 1) * 128], s, idn[:])
            nc.vector.tensor_copy(dst, pq[:])

        def stage_xT(blk, b):
            xT = xTp.tile([128, 2, L], BF, tag="xT")
            for q in range(2):
                xb = (xb_tiles[blk, b, q] if BF16_TRANSPOSE
                      else xt_tiles[blk, b, q])
                for ec in range(2):
                    transpose_quad(
                        xT[:, ec, q * 512:(q + 1) * 512],
                        [xb[:, i, ec * 128:(ec + 1) * 128] for i in range(4)],
                        f32=not BF16_TRANSPOSE,
                    )
            xT_tiles[blk] = xT

        def stage_prep(blk, b):
            xT = xT_tiles[blk]
            for n in range(NH):
                for kq in ("k", "q"):
                    ps = psS.tile([128, L], F32, tag="S")
                    for qc in range(2):
                        sl = slice(qc * 512, (qc + 1) * 512)
                        for ec in range(2):
                            nc.tensor.matmul(
                                ps[:, sl], lhsT=w_sb[blk, n, kq, ec][:],
                                rhs=xT[:, ec, sl],
                                start=(ec == 0), stop=(ec == 1),
                            )
                    t = kqp.tile([128, L], BF, tag="kq")
                    nc.scalar.activation(t[:], ps[:], AF.Sigmoid)
                    kqT[blk, n, kq] = t

        def stage_kx(blk, b):
            for n in range(NH):
                kx = kxp.tile([128, L], BF, tag="kx")
                if BF16_TRANSPOSE:
                    src = kqT[blk, n, "k"]
                    for q in range(2):
                        transpose_quad(
                            kx[:, q * 512:(q + 1) * 512],
                            [src[:, (q * 4 + i) * 128:(q * 4 + i + 1) * 128]
                             for i in range(4)],
                        )
                else:
                    # recompute x@w in [l, f] orientation (no bf16 psum)
                    xT = xT_tiles[blk]
                    pf = psU.tile([128, L], F32, tag="U")
                    for lc in range(8):
                        for ec in range(2):
                            nc.tensor.matmul(
                                pf[:, lc * 128:(lc + 1) * 128],
                                lhsT=xT[:, ec, lc * 128:(lc + 1) * 128],
                                rhs=w_sb[blk, n, "k", ec][:],
                                start=(ec == 0), stop=(ec == 1),
                                skip_group_check=True,
                            )
                    nc.scalar.activation(kx[:], pf[:], AF.Sigmoid)
                kx_t[blk, n] = kx

        def stage_heads(blk, b):
            for n in range(NH):
                kq_k = kqT[blk, n, "k"]
                kq_q = kqT[blk, n, "q"]
                kx = kx_t[blk, n]
                pU = psU.tile([128, L], F32, tag="U")
                acc = accp.tile([128, L], BF, tag="acc")
                ex_prev = None
                for kc in range(8):
                    pS = psS.tile([128, L], F32, tag="S")
                    for qc in range(2):
                        sl = slice(qc * 512, (qc + 1) * 512)
                        nc.tensor.matmul(
                            pS[:, sl], lhsT=kq_k[:, kc * 128:(kc + 1) * 128],
                            rhs=kq_q[:, sl], start=True, stop=True,
                        )
                    ex = expp.tile([128, L], BF, tag="ex")
                    nc.scalar.activation(ex[:], pS[:], AF.Exp, scale=SCALE)
                    if kc == 1:
                        nc.vector.tensor_add(acc[:], ex_prev[:], ex[:])
                    elif kc > 1:
                        nc.vector.tensor_add(acc[:], acc[:], ex[:])
                    ex_prev = ex
                    for qc in range(2):
                        sl = slice(qc * 512, (qc + 1) * 512)
                        nc.tensor.matmul(
                            pU[:, sl], lhsT=kx[:, kc * 128:(kc + 1) * 128],
                            rhs=ex[:, sl], start=(kc == 0), stop=(kc == 7),
                            skip_group_check=True,
                        )
                pSum = psS.tile([128, L], F32, tag="S")
                for qc in range(2):
                    sl = slice(qc * 512, (qc + 1) * 512)
                    nc.tensor.matmul(
                        pSum[:, sl], lhsT=ones_bf[:], rhs=acc[:, sl],
                        start=True, stop=True,
                    )
                rec = nrmp.tile([128, L], F32, tag="rec")
                if USE_FAST_RECIP:
                    nc.vector.reciprocal_approx_fast(out=rec[:], in_=pSum[:])
                else:
                    nc.vector.reciprocal(rec[:], pSum[:])
                nU = nUp.tile([128, L], BF, tag="nU")
                nc.vector.tensor_mul(nU[:], pU[:], rec[:])
                normU[blk, n] = nU

        def stage_proj(blk, b):
            for oc in range(2):
                pF = psS.tile([128, L], F32, tag="S")
                for qc in range(2):
                    sl = slice(qc * 512, (qc + 1) * 512)
                    for fc in range(2):
                        nc.tensor.matmul(
                            pF[:, sl],
                            lhsT=Wp_sb[blk, fc][:, oc * 128:(oc + 1) * 128],
                            rhs=normU[blk, fc][:, sl],
                            start=(fc == 0), stop=(fc == 1),
                        )
                so = selfout.tile([128, L], BF, tag="so")
                nc.vector.tensor_scalar(
                    out=so[:], in0=pF[:], scalar1=bp_sb[blk, oc][:],
                    scalar2=None, op0=OP.add,
                )
                so_t[blk, b, oc] = so

        def mattn_prep(b, side):
            blk = "d" if side == "k" else "p"
            wsb = wkm_sb if side == "k" else wqm_sb
            for fc in range(2):
                ps = psS.tile([128, L], F32, tag="S")
                for lc2 in range(2):
                    sl = slice(lc2 * 512, (lc2 + 1) * 512)
                    for ec in range(2):
                        nc.tensor.matmul(
                            ps[:, sl], lhsT=wsb[ec][:, fc * 128:(fc + 1) * 128],
                            rhs=so_t[blk, b, ec][:, sl],
                            start=(ec == 0), stop=(ec == 1),
                        )
                km = matp.tile([128, L], BF, tag="km")
                nc.scalar.activation(km[:], ps[:], AF.Sigmoid)
                kmT[side, fc] = km

        def mattn_mid(b):
            pRow = psS.tile([128, L], F32, tag="S")
            for fc in range(2):
                mm = kmT["k", fc]
                nc.vector.tensor_mul(mm[:], kmT["k", fc][:], kmT["q", fc][:])
                for lc2 in range(2):
                    sl = slice(lc2 * 512, (lc2 + 1) * 512)
                    nc.tensor.matmul(
                        pRow[0:1, sl], lhsT=Wm_col[fc][:], rhs=mm[:, sl],
                        start=(fc == 0), stop=(fc == 1), skip_group_check=True,
                    )
            return pRow

        def mattn_tail(b, pRow):
            sig = smallp.tile([1, L], F32, tag="sig")
            nc.scalar.activation(sig[:], pRow[0:1, :], AF.Sigmoid, bias=bm_sb[:])
            erow = smallp.tile([1, L], BF, tag="erow")
            T11 = smallp.tile([1, 1], F32, tag="t11")
            nc.scalar.activation(
                erow[:], sig[:], AF.Exp, scale=MSCALE, accum_out=T11[:]
            )
            pT = psS.tile([128, L], F32, tag="S")
            nc.tensor.matmul(
                pT[:, 0:1], lhsT=ones_f[0:1, :], rhs=T11[:], start=True, stop=True,
            )
            rT = smallp.tile([128, 1], F32, tag="rt")
            nc.vector.reciprocal(rT[:], pT[:, 0:1])
            pB = psS.tile([128, L], F32, tag="S")
            for lc2 in range(2):
                sl = slice(lc2 * 512, (lc2 + 1) * 512)
                nc.tensor.matmul(
                    pB[:, sl], lhsT=ones_bf[0:1, :], rhs=erow[:, sl],
                    start=True, stop=True,
                )
            cn2 = smallp.tile([128, 1, 2], F32, tag="cn2")
            junks, css = [], []
            for oc in range(2):
                junk = nrmp.tile([128, L], BF, tag="junk")
                nc.vector.tensor_mul(junk[:], so_t["d", b, oc][:], pB[:])
                junks.append(junk)
            sink = accp.tile([128, L], BF, tag="acc")
            for oc in range(2):
                # free-axis sum on ACT (Copy + accum_out); ACT is idle here
                cs = smallp.tile([128, 1], F32, tag="cs")
                nc.scalar.activation(
                    sink[:], junks[oc][:], AF.Copy, accum_out=cs[:])
                css.append(cs)
            for oc in range(2):
                nc.vector.tensor_scalar(
                    out=cn2[:, 0, oc:oc + 1], in0=css[oc][:], scalar1=rT[:],
                    scalar2=None, op0=OP.mult,
                )
            nc.gpsimd.dma_start(
                out_t[b, 0:1, :].rearrange("a (oc p) -> p a oc", p=128), cn2[:])

        # ---- pipeline emission ----
        def batch_front(b):
            stage_xT("d", b)
            stage_prep("d", b)
            stage_xT("p", b)
            stage_prep("p", b)
            stage_kx("d", b)
            stage_kx("p", b)

        def batch_mid(b):
            stage_heads("d", b)
            stage_heads("p", b)

        def batch_back(b):
            stage_proj("d", b)
            mattn_prep(b, "k")
            stage_proj("p", b)
            mattn_prep(b, "q")
            return mattn_mid(b)

        batch_front(0)
        batch_mid(0)
        for blk in ("d", "p"):
            for half in range(2):
                issue_xload(blk, 1, half)
        cast_x("d", 1)
        cast_x("p", 1)
        pRow0 = batch_back(0)
        batch_front(1)
        mattn_tail(0, pRow0)
        batch_mid(1)
        pRow1 = batch_back(1)
        mattn_tail(1, pRow1)

    nc.compile()
    return nc


_CACHE = {}


def _get_nc():
    if "nc" not in _CACHE:
        _CACHE["nc"] = build()
    return _CACHE["nc"]


def _make_in_maps(inputs):
    in_maps = []
    for c in range(CORES):
        m = {}
        sl = slice(c * BPC, (c + 1) * BPC)
        m["x_d"] = np.ascontiguousarray(np.asarray(inputs["x_d"], np.float32)[sl])
        m["x_p"] = np.ascontiguousarray(np.asarray(inputs["x_p"], np.float32)[sl])
        for name in WNAMES:
            m[name] = np.ascontiguousarray(np.asarray(inputs[name], np.float32))
        in_maps.append(m)
    return in_maps


def run_spmd(inputs, **kw):
    nc = _get_nc()
    res = run_bass_kernel_spmd(nc, _make_in_maps(inputs), core_ids=list(range(CORES)), **kw)
    out = np.concatenate([r["out"] for r in res.results], axis=0)
    return out, res


def kernel(**inputs):
    out, _ = run_spmd(inputs)
    return out
